# revision 1
# baseline (speedup 1.0000x reference)
"""Trainium2 Bass kernel for the CMA momentum-memory update (nn_CMA_52956946760162).

Strategy (class-sharded, present-only compact packing):
- Shard the C=4096 classes across 8 cores (512 classes/core), no collectives.
- Host packs, per (core, modality), the *present* (label,cam) segments and
  present labels into chunks of <=128 one-hot columns / <=128*B feature rows
  (whole classes per chunk). The one-hot entries are pre-scaled with the
  momentum/count coefficients (b_c = sigma_or_1/cnt, b_g = sigma/cnt), and a
  segment column and its class column share the same matmul, so one tensor-
  engine pass produces both per-(label,cam) and per-label scaled sums in PSUM.
- Host gathers the corresponding memory-bank rows densely (mem_in), so every
  device DMA is a dense [128 x 2048] f32 block. The device computes
  out = a * mem + psum in a single fused DVE op per chunk and streams it out.
- Rows absent from the batch leave memory unchanged; the host passes them
  through directly from the input banks during output assembly and scatters
  the device-computed rows over them.
"""

import numpy as np

C, K, D, N = 4096, 6, 2048, 16384
SIGMA = 0.2
M = 8                 # cores
CPC = C // M          # classes per core = 512
CK = C * K
F32 = np.float32

_BUILD_CACHE = {}


def _pack_core_modality(core, feats, labels, cams, valid, B, nch):
    """Pack one (core, modality) into chunk tensors.

    Returns fpad [nch*B*128, D], oh [nch, B*128, 128], avec [128, nch],
    mem_idx/out_idx [nch, 128] (merged row id: class c -> c, seg s -> CPC + s,
    pad -> -1).
    """
    c0 = core * CPC
    mask = (labels >= c0) & (labels < c0 + CPC)
    rows_all = np.nonzero(mask)[0]
    lab = labels[rows_all] - c0
    seg = lab * K + cams[rows_all]
    order = np.argsort(seg, kind="stable")
    rows_all, lab, seg = rows_all[order], lab[order], seg[order]

    ccnt = np.bincount(seg, minlength=CPC * K).astype(F32)
    gcnt = np.bincount(lab, minlength=CPC).astype(F32)
    v = np.asarray(valid[c0:c0 + CPC]).reshape(CPC * K)
    a_c = np.where(v, 1.0 - SIGMA, 0.0).astype(F32)
    b_c = (np.where(v, SIGMA, 1.0) / np.maximum(ccnt, 1.0)).astype(F32)
    b_g = (SIGMA / np.maximum(gcnt, 1.0)).astype(F32)

    cpres = ccnt > 0
    class_start = np.searchsorted(lab, np.arange(CPC + 1))
    nseg_per_class = cpres.reshape(CPC, K).sum(axis=1)

    chunk_id = np.empty(len(rows_all), np.int64)
    slot = np.empty(len(rows_all), np.int64)
    segcol_of = np.empty(CPC * K, np.int64)
    ccol_of = np.empty(CPC, np.int64)
    mem_idx = np.full((nch, 128), -1, np.int64)
    out_idx = np.full((nch, 128), -1, np.int64)
    avec = np.zeros((128, nch), F32)

    present = np.nonzero(gcnt > 0)[0]
    chunk_classes = []
    cur, cols, rws = [], 0, 0
    for c in present:
        ns = int(nseg_per_class[c])
        nr = int(class_start[c + 1] - class_start[c])
        if cur and (cols + ns + 1 > 128 or rws + nr > B * 128):
            chunk_classes.append(cur)
            cur, cols, rws = [], 0, 0
        cur.append(c)
        cols += ns + 1
        rws += nr
    if cur:
        chunk_classes.append(cur)
    assert len(chunk_classes) <= nch

    for j, cls_list in enumerate(chunk_classes):
        lo, rws = 0, 0
        for c in cls_list:
            segs_c = np.nonzero(cpres[c * K:(c + 1) * K])[0] + c * K
            for s in segs_c:
                p, lo = lo, lo + 1
                segcol_of[s] = p
                out_idx[j, p] = CPC + s
                avec[p, j] = a_c[s]
                mem_idx[j, p] = CPC + s
            p, lo = lo, lo + 1
            ccol_of[c] = p
            out_idx[j, p] = c
            avec[p, j] = 1.0 - SIGMA
            mem_idx[j, p] = c
            r0, r1 = int(class_start[c]), int(class_start[c + 1])
            chunk_id[r0:r1] = j
            slot[r0:r1] = rws + np.arange(r1 - r0)
            rws += r1 - r0
        assert lo <= 128

    fpoh = np.zeros((nch, B * 128, D + 128), F32)
    fpoh[chunk_id, slot, :D] = feats[rows_all]
    fpoh[chunk_id, slot, D + segcol_of[seg]] = b_c[seg]
    fpoh[chunk_id, slot, D + ccol_of[lab]] = b_g[lab]
    return dict(fpoh=fpoh.reshape(nch * B * 128, D + 128), avec=avec,
                mem_idx=mem_idx, out_idx=out_idx)


def _chunk_stats(labels, cams, valid):
    """Per core: (max rows per class, gcnt, n1_of, n0_of)."""
    out = []
    for core in range(M):
        c0 = core * CPC
        mask = (labels >= c0) & (labels < c0 + CPC)
        lab = labels[mask] - c0
        seg = lab * K + cams[mask]
        gcnt = np.bincount(lab, minlength=CPC)
        cpres = np.bincount(seg, minlength=CPC * K) > 0
        v = np.asarray(valid[c0:c0 + CPC]).reshape(CPC * K)
        vseg = (cpres & v).reshape(CPC, K).sum(axis=1)
        nseg = cpres.reshape(CPC, K).sum(axis=1)
        out.append((int(gcnt.max()), gcnt, vseg + 1, nseg - vseg))
    return out


def _count_chunks(gcnt, n1_of, n0_of, B):
    j, cols, rws, any_rows = 0, 0, 0, False
    for c in np.nonzero(gcnt > 0)[0]:
        ns = int(n1_of[c] + n0_of[c])    # total cols for class c
        nr = int(gcnt[c])
        if any_rows and (cols + ns > 128 or rws + nr > B * 128):
            j += 1
            cols, rws = 0, 0
        cols += ns
        rws += nr
        any_rows = True
    return j + 1 if any_rows else 0


def _build_program(B, nch):
    """Build + compile the SPMD Bass program; 2*nch chunks (both modalities)."""
    import concourse.mybir as mybir
    import concourse.tile as tile
    from concourse import bacc

    f32 = mybir.dt.float32
    nc = bacc.Bacc("TRN2", target_bir_lowering=False, debug=False)

    NT = 2 * nch
    H = D // 2
    fpoh = nc.dram_tensor("fpoh", [NT * B * 128, D + 128], f32, kind="ExternalInput").ap()
    memin = nc.dram_tensor("memin", [NT * 128, D], f32, kind="ExternalInput").ap()
    avec = nc.dram_tensor("avec", [128, NT], f32, kind="ExternalInput").ap()
    out = nc.dram_tensor("out", [NT * 128, D], f32, kind="ExternalOutput").ap()

    with tile.TileContext(nc) as tc:
        with tc.tile_pool(name="const", bufs=1) as constp, \
             tc.tile_pool(name="io", bufs=6) as iop, \
             tc.tile_pool(name="ps", bufs=2, space="PSUM") as psp:

            avec_t = constp.tile([128, NT], f32, name="avec_t")
            nc.sync.dma_start(out=avec_t[:], in_=avec[:, :])

            for j in range(NT):
                psum = psp.tile([128, D], f32, tag="ps", name="psum")
                for b in range(B):
                    r0 = (j * B + b) * 128
                    frow = iop.tile([128, D + 128], f32, tag="frow", name="frow")
                    nc.sync.dma_start(out=frow[:], in_=fpoh[r0:r0 + 128, :])
                    for t in range(4):
                        sl = slice(t * 512, (t + 1) * 512)
                        nc.tensor.matmul(psum[:, sl], frow[:, D:D + 128], frow[:, sl],
                                         start=(b == 0), stop=(b == B - 1))
                mem_sb = iop.tile([128, D], f32, tag="mem", bufs=5, name="mem_sb")
                nc.scalar.dma_start(out=mem_sb[:], in_=memin[j * 128:(j + 1) * 128, :])
                out_sb = iop.tile([128, D], f32, tag="out", bufs=8, name="out_sb")
                nc.vector.scalar_tensor_tensor(
                    out=out_sb[:], in0=mem_sb[:], scalar=avec_t[:, j:j + 1],
                    in1=psum[:], op0=mybir.AluOpType.mult, op1=mybir.AluOpType.add)
                nc.gpsimd.dma_start(out=out[j * 128:(j + 1) * 128, :], in_=out_sb[:])

    nc.compile()
    return nc


def prepare(inputs):
    """Build (or reuse) the program and the per-core input maps + scatter metadata."""
    a = {k: np.ascontiguousarray(np.asarray(v)) for k, v in inputs.items()}
    mods = [
        (a["rgb_feats"], a["rgb_labels"].astype(np.int64), a["rgb_cams"].astype(np.int64),
         a["vis_cam_valid"], a["vis_memory"], a["vis_cam_memory"].reshape(CK, D)),
        (a["ir_feats"], a["ir_labels"].astype(np.int64), a["ir_cams"].astype(np.int64),
         a["ir_cam_valid"], a["ir_memory"], a["ir_cam_memory"].reshape(CK, D)),
    ]

    # global B and chunk count (uniform across cores -> one SPMD program)
    B = 1
    stats = []
    for feats, labels, cams, valid, gmem, cmem in mods:
        st = _chunk_stats(labels, cams, valid)
        stats.append(st)
        for mx, _, _, _ in st:
            B = max(B, int(np.ceil(mx / 128)))
    nch = 1
    for st in stats:
        for _, gcnt, n1_of, n0_of in st:
            nch = max(nch, _count_chunks(gcnt, n1_of, n0_of, B))

    key = (B, nch)
    if key not in _BUILD_CACHE:
        _BUILD_CACHE[key] = _build_program(B, nch)
    nc = _BUILD_CACHE[key]

    in_maps, metas = [], []
    for core in range(M):
        c0 = core * CPC
        packs = []
        for m, (feats, labels, cams, valid, gmem, cmem) in enumerate(mods):
            packs.append(_pack_core_modality(core, feats, labels, cams, valid, B, nch))
        im = {
            "fpoh": np.concatenate([p["fpoh"] for p in packs], axis=0),
            "avec": np.concatenate([p["avec"] for p in packs], axis=1),
        }
        memin = np.zeros((2 * nch * 128, D), F32)
        meta = []
        for m, p in enumerate(packs):
            gmem, cmem = mods[m][4], mods[m][5]
            idx = p["mem_idx"].reshape(nch * 128)
            used = np.nonzero(idx >= 0)[0]
            gidx = idx[used]
            isg = gidx < CPC
            src = np.where(isg, c0 + gidx, core * CPC * K + (gidx - CPC))
            block = memin[m * nch * 128:(m + 1) * nch * 128]
            block[used[isg]] = gmem[src[isg]]
            block[used[~isg]] = cmem[src[~isg]]
            oidx = p["out_idx"].reshape(nch * 128)
            oused = np.nonzero(oidx >= 0)[0]
            ogidx = oidx[oused]
            oisg = ogidx < CPC
            obase = (C + CK) * m
            tgt = np.where(oisg, obase + c0 + ogidx,
                           obase + C + core * CPC * K + (ogidx - CPC))
            meta.append((oused + m * nch * 128, tgt))
        im["memin"] = memin
        in_maps.append(im)
        metas.append(meta)
    return nc, in_maps, metas, a, mods


def assemble(a, mods, metas, results):
    full = np.concatenate([a["vis_memory"], mods[0][5], a["ir_memory"], mods[1][5]],
                          axis=0).astype(F32, copy=True)
    for core in range(M):
        o = results[core]["out"]
        for used, tgt in metas[core]:
            full[tgt] = o[used]
    return full


def kernel(**inputs):
    from concourse.bass_utils import run_bass_kernel_spmd

    nc, in_maps, metas, a, mods = prepare(inputs)
    res = run_bass_kernel_spmd(nc, in_maps, core_ids=list(range(M)))
    return assemble(a, mods, metas, res.results)



# revision 2
# speedup vs baseline: 1.0935x; 1.0935x over previous
"""Trainium2 Bass kernel v5 for the CMA momentum-memory update.

kernel_v3 (bf16 hi/lo matmul, metadata one-hots, host-prescaled memory rows)
plus DMA granularity changes aimed at descriptor overhead:
- chunks are processed in PAIRS: one fm/memin/out DMA per pair moves two
  chunks' rows per partition (16KB descriptors instead of 8KB, half the
  descriptor count per engine);
- memin is a uniform full-128-partition transfer again (rows beyond the
  chunk's mem columns are zeros) so every transfer loads all 16 SDMA engines
  with identical 8-row shares, like the baseline's traffic pattern.
"""

import numpy as np

C, K, D, N = 4096, 6, 2048, 16384
SIGMA = 0.2
M = 8
CPC = C // M
CK = C * K
F32 = np.float32
FMB_W = 2 * D + 16        # bf16 cols per chunk row: hi | lo | meta+pad (8224B)
VMEM = 128                # memin rows per chunk (full partitions, zero-padded)
GRP = 2                   # chunks per DMA group (16KB descriptors)

_BUILD_CACHE = {}


def _rne_bf16_u16(x):
    u = np.ascontiguousarray(x, np.float32).view(np.uint32)
    return ((u + 0x7FFF + ((u >> 16) & 1)) >> 16).astype(np.uint16)


def _u16_to_f32(h):
    return (h.astype(np.uint32) << 16).view(np.float32)


def _core_items(core, mods):
    items = []
    c0 = core * CPC
    for m, md in enumerate(mods):
        labels, cams = md["labels"], md["cams"]
        sel = np.nonzero((labels >= c0) & (labels < c0 + CPC))[0]
        lab = labels[sel] - c0
        cam = cams[sel]
        order = np.argsort(lab * K + cam, kind="stable")
        sel, lab, cam = sel[order], lab[order], cam[order]
        cls_ids, cls_start = np.unique(lab, return_index=True)
        cls_start = np.append(cls_start, len(lab))
        vmat = md["valid"]
        for ci, c in enumerate(cls_ids):
            r0, r1 = int(cls_start[ci]), int(cls_start[ci + 1])
            crows, ccam = sel[r0:r1], cam[r0:r1]
            u, ustart = np.unique(ccam, return_index=True)
            ustart = np.append(ustart, len(ccam))
            seglist, nmem, noth = [], 1, 0
            for si, cm in enumerate(u):
                isv = bool(vmat[c0 + c, cm])
                seglist.append((int(cm), isv, crows[ustart[si]:ustart[si + 1]]))
                nmem, noth = nmem + isv, noth + (not isv)
            items.append((r1 - r0, nmem, noth, (m, int(c0 + c), seglist)))
    return items


def _pack_bins(items, NCH):
    tot = np.array([(it[0], it[1], it[2]) for it in items]).sum(axis=0)
    tgt = np.maximum(tot / NCH, 1.0)
    fill = [[0, 0, 0] for _ in range(NCH)]
    bins = [[] for _ in range(NCH)]
    for it in sorted(items, key=lambda x: (-x[0], -x[1])):
        best, bestscore = -1, -1e18
        for k in range(NCH):
            bn = fill[k]
            if (bn[0] + it[0] <= 128
                    and bn[1] + it[1] + bn[2] + it[2] <= 128):
                s = (tgt[0] - bn[0]) / tgt[0] + (tgt[1] - bn[1]) / tgt[1] \
                    + (tgt[2] - bn[2]) / tgt[2]
                if s > bestscore:
                    bestscore, best = s, k
        if best < 0:
            return None
        fill[best][0] += it[0]
        fill[best][1] += it[1]
        fill[best][2] += it[2]
        bins[best].append(it)
    return bins


def _choose_nch(all_items):
    NCH = 32
    while NCH < 100:
        if all(_pack_bins(it, NCH) is not None for it in all_items):
            return NCH
        NCH += 1
    raise AssertionError("packing infeasible")


def _fill_core(core, mods, bins, NCH):
    fm = np.zeros((NCH * 128, FMB_W), np.uint16)
    memin = np.zeros((NCH * VMEM, D), F32)
    bvec = np.zeros((128, NCH), F32)
    src, tgt = [], []
    rsrc = [[] for _ in mods]
    rdst = [[] for _ in mods]
    meta = np.zeros((NCH * 128, 2), F32)
    for j, bn in enumerate(bins):
        mcur, ocur, rowc = 0, 0, 0
        nmem_tot = sum(it[1] for it in bn)
        for (nr, nmem, noth, (m, gc, seglist)) in bn:
            md = mods[m]
            base_out = m * (C + CK)

            def take_mem():
                nonlocal mcur
                p = mcur
                mcur += 1
                return p

            def take_oth():
                nonlocal ocur
                p = nmem_tot + ocur
                ocur += 1
                return p

            ccol = take_mem()
            memin[j * VMEM + ccol] = (1.0 - SIGMA) * md["gmem"][gc]
            bvec[ccol, j] = SIGMA / nr
            src.append(j * 128 + ccol)
            tgt.append(base_out + gc)
            for (cm, isv, s_rows) in seglist:
                cnt = len(s_rows)
                if isv:
                    col = take_mem()
                    bvec[col, j] = SIGMA / cnt
                    memin[j * VMEM + col] = (1.0 - SIGMA) * md["cmem"][gc * K + cm]
                else:
                    col = take_oth()
                    bvec[col, j] = 1.0 / cnt
                src.append(j * 128 + col)
                tgt.append(base_out + C + gc * K + cm)
                rr = j * 128 + rowc
                rowc += cnt
                rsrc[m].append(s_rows)
                rdst[m].append(np.arange(rr, rr + cnt))
                meta[rr:rr + cnt, 0] = col
                meta[rr:rr + cnt, 1] = ccol
        assert rowc <= 128 and nmem_tot + ocur <= 128
    for m, md in enumerate(mods):
        if rsrc[m]:
            s = np.concatenate(rsrc[m])
            d = np.concatenate(rdst[m])
            x = md["feats"][s]
            hi = _rne_bf16_u16(x)
            lo = _rne_bf16_u16(x - _u16_to_f32(hi))
            fm[d, 0:D] = hi
            fm[d, D:2 * D] = lo
    fm[:, 2 * D:2 * D + 4] = meta.view(np.uint16).reshape(NCH * 128, 4)
    # group-interleave: DRAM row p of group g = chunk(GRP*g+0..) rows p side by side
    NG = (NCH + GRP - 1) // GRP
    fmP = np.zeros((NG * 128, GRP * FMB_W), np.uint16)
    memP = np.zeros((NG * 128, GRP * D), F32)
    for g in range(NG):
        for cj in range(GRP):
            j = GRP * g + cj
            if j >= NCH:
                break
            fmP[g * 128:(g + 1) * 128, cj * FMB_W:(cj + 1) * FMB_W] = \
                fm[j * 128:(j + 1) * 128]
            memP[g * 128:(g + 1) * 128, cj * D:(cj + 1) * D] = \
                memin[j * VMEM:(j + 1) * VMEM]
    return fmP, memP, bvec, \
        (np.asarray(src, np.int64), np.asarray(tgt, np.int64))


def _build_program(NCH):
    import concourse.mybir as mybir
    import concourse.tile as tile
    from concourse import bacc

    f32 = mybir.dt.float32
    bf16 = mybir.dt.bfloat16
    u16 = mybir.dt.uint16
    alu = mybir.AluOpType
    nc = bacc.Bacc("TRN2", target_bir_lowering=False, debug=False)

    NG = (NCH + GRP - 1) // GRP
    fm = nc.dram_tensor("fm", [NG * 128, GRP * FMB_W], u16, kind="ExternalInput").ap()
    memin = nc.dram_tensor("memin", [NG * 128, GRP * D], f32, kind="ExternalInput").ap()
    bvec = nc.dram_tensor("bvec", [128, NCH], f32, kind="ExternalInput").ap()
    iota = nc.dram_tensor("iota", [128, 128], f32, kind="ExternalInput").ap()
    out = nc.dram_tensor("out", [NG * 128, GRP * D], f32, kind="ExternalOutput").ap()

    with tile.TileContext(nc) as tc:
        with tc.tile_pool(name="const", bufs=1) as constp, \
             tc.tile_pool(name="io", bufs=3) as iop, \
             tc.tile_pool(name="ohp", bufs=3) as ohp, \
             tc.tile_pool(name="ps", bufs=2, space="PSUM") as psp:

            iota_t = constp.tile([128, 128], f32, name="iota_t")
            bvec_t = constp.tile([128, NCH], f32, name="bvec_t")

            first = True
            for g in range(NG):
                chunks = [GRP * g + c for c in range(GRP) if GRP * g + c < NCH]
                W = len(chunks) * FMB_W
                frow = iop.tile([128, GRP * FMB_W], u16, tag="frow", name="frow")
                nc.sync.dma_start(out=frow[:, 0:W],
                                  in_=fm[g * 128:(g + 1) * 128, 0:W])
                if first:
                    nc.sync.dma_start(out=iota_t[:], in_=iota[:, :])
                    nc.sync.dma_start(out=bvec_t[:], in_=bvec[:, :])
                    first = False

                mem_sb = iop.tile([128, GRP * D], f32, tag="mem", name="mem_sb")
                nc.gpsimd.dma_start(
                    out=mem_sb[:, 0:len(chunks) * D],
                    in_=memin[g * 128:(g + 1) * 128, 0:len(chunks) * D])

                psums = []
                for cj, j in enumerate(chunks):
                    off = cj * FMB_W
                    metaf = frow[:, off + 2 * D:off + 2 * D + 4].bitcast(f32)
                    t1 = ohp.tile([128, 128], bf16, tag="t1", name="t1")
                    t2 = ohp.tile([128, 128], bf16, tag="t2", name="t2")
                    oh = ohp.tile([128, 128], bf16, tag="oh", name="oh")
                    nc.vector.tensor_scalar(
                        out=t1[:], in0=iota_t[:], scalar1=metaf[:, 0:1],
                        scalar2=None, op0=alu.is_equal)
                    nc.vector.tensor_scalar(
                        out=t2[:], in0=iota_t[:], scalar1=metaf[:, 1:2],
                        scalar2=None, op0=alu.is_equal)
                    nc.vector.tensor_tensor(out=oh[:], in0=t1[:], in1=t2[:],
                                            op=alu.add)
                    psum = psp.tile([128, D], f32, tag="ps", name="psum")
                    for t in range(4):
                        sl = slice(t * 512, (t + 1) * 512)
                        hi = frow[:, off + t * 512:off + (t + 1) * 512].bitcast(bf16)
                        lo = frow[:, off + D + t * 512:off + D + (t + 1) * 512].bitcast(bf16)
                        nc.tensor.matmul(psum[:, sl], oh[:], hi,
                                         start=True, stop=False)
                        nc.tensor.matmul(psum[:, sl], oh[:], lo,
                                         start=False, stop=True)
                    psums.append(psum)

                out_sb = iop.tile([128, GRP * D], f32, tag="out", name="out_sb")
                for cj, j in enumerate(chunks):
                    nc.vector.scalar_tensor_tensor(
                        out=out_sb[:, cj * D:(cj + 1) * D], in0=psums[cj][:],
                        scalar=bvec_t[:, j:j + 1],
                        in1=mem_sb[:, cj * D:(cj + 1) * D],
                        op0=alu.mult, op1=alu.add)
                nc.scalar.dma_start(
                    out=out[g * 128:(g + 1) * 128, 0:len(chunks) * D],
                    in_=out_sb[:, 0:len(chunks) * D])

    nc.compile()
    return nc


def prepare(inputs):
    a = {k: np.ascontiguousarray(np.asarray(v)) for k, v in inputs.items()}
    mods = [
        dict(feats=a["rgb_feats"], labels=a["rgb_labels"].astype(np.int64),
             cams=a["rgb_cams"].astype(np.int64), valid=np.asarray(a["vis_cam_valid"]),
             gmem=a["vis_memory"], cmem=a["vis_cam_memory"].reshape(CK, D)),
        dict(feats=a["ir_feats"], labels=a["ir_labels"].astype(np.int64),
             cams=a["ir_cams"].astype(np.int64), valid=np.asarray(a["ir_cam_valid"]),
             gmem=a["ir_memory"], cmem=a["ir_cam_memory"].reshape(CK, D)),
    ]

    all_items = [_core_items(core, mods) for core in range(M)]
    NCH = _choose_nch(all_items)

    if NCH not in _BUILD_CACHE:
        _BUILD_CACHE[NCH] = _build_program(NCH)
    nc = _BUILD_CACHE[NCH]

    iota_const = np.broadcast_to(np.arange(128, dtype=F32), (128, 128)).copy()
    in_maps, scatters = [], []
    for core in range(M):
        bins = _pack_bins(all_items[core], NCH)
        fm, memin, bvec, scat = _fill_core(core, mods, bins, NCH)
        in_maps.append({"fm": fm, "memin": memin, "bvec": bvec,
                        "iota": iota_const})
        scatters.append(scat)
    return nc, in_maps, scatters, a, mods


def assemble(a, mods, scatters, results):
    full = np.concatenate([a["vis_memory"], mods[0]["cmem"],
                           a["ir_memory"], mods[1]["cmem"]],
                          axis=0).astype(F32, copy=True)
    for core in range(M):
        src, tgt = scatters[core]
        o = results[core]["out"]
        NGr = o.shape[0] // 128
        # group layout [NG*128, GRP*D] -> chunk-major [NG*GRP*128, D]
        oc = o.reshape(NGr, 128, GRP, D).transpose(0, 2, 1, 3).reshape(NGr * GRP * 128, D)
        full[tgt] = oc[src]
    return full


def src_to_pair(src):
    # src indexes chunk-major rows (j*128 + p); pair-major row for chunk j is
    # (j//2)*2*128 + (j%2)*128 + p  == same ordering -> identity
    return src


def kernel(**inputs):
    from concourse.bass_utils import run_bass_kernel_spmd

    nc, in_maps, scatters, a, mods = prepare(inputs)
    res = run_bass_kernel_spmd(nc, in_maps, core_ids=list(range(M)))
    return assemble(a, mods, scatters, res.results)
